# revision 29
# baseline (speedup 1.0000x reference)
"""Cumulative LayerNorm (B=4, C=512, T=32000) on 8 Trainium2 NeuronCores.

Sharding: core j handles batch b = j//2 and T-half h = j%2 (16000 time
steps), ALL 512 channels. Per-t channel sums are complete locally, so
the only cross-core data is each half's grand total (2 f32 scalars):
ONE 16-byte AllReduce per kernel, vs. the channel-split design's 128KB
through the ~2GB/s CC engine (which measured ~10us/op serialized and
paced the whole kernel).

x is fed bf16 and y stored bf16 (upcast on host).

Structure per core:
  Phase 1 (pipelined over 5 x 3200-t segments): load | squares
    (ScalarE 2cb + DVE 2cb) | per-t sums via one-hot-column matmuls
    into [8,400] PSUM banks | PSUM export | local t-major scan with
    running carry (offsets via strict-triangular PE matmul).
  Boundary: final carry -> masked [1,4] f32 AllReduce (even core's
    totals land in slots 0:2) -> os = r4[0:2]*h -> PE-broadcast os to
    all 128 partitions.
  Phase 2 (pipelined): finalize A/B per segment -- the cross-core
    offset folds into the existing ops via scalar_tensor_tensor
    ((cum+os)*inv) at zero extra cost -- then flat 128-partition
    stride-0 fan of the A/B rows, y = x*A + B (DVE 3cb + GpSimd 1cb),
    store.
"""
import numpy as np

import concourse.bass as bass
import concourse.bacc as bacc
import concourse.tile as tile
from concourse import mybir
from concourse.bass_utils import run_bass_kernel_spmd

F32 = mybir.dt.float32
F32R = mybir.dt.float32r
BF16 = mybir.dt.bfloat16

B, C, T = 4, 512, 32000
NCORES = 8
TH = T // 2          # 16000 t per core
CB = C // 128        # 4 channel blocks
SEG = 3200           # segment length along T
NSEG = TH // SEG     # 5
F = SEG // 128       # 25 (t-major free dim per segment)
TS = 400             # stats matmul tile (moving cols)
NTS = SEG // TS      # 8
HALF = SEG // 2
EPS = 1e-08
RG = [[0, 1], [2, 3], [4, 5], [6, 7]]  # batch-pair replica groups

_CACHE = {}


def _build(wb_general: bool):
    nc = bacc.Bacc()

    xc_e = nc.declare_dram_parameter("xc", [C, TH], BF16, isOutput=False)
    tri_e = nc.declare_dram_parameter("tri", [128, 128], F32R, isOutput=False)
    invp_e = nc.declare_dram_parameter("invp", [128, F * NSEG], F32, isOutput=False)
    invm_e = nc.declare_dram_parameter("invm", [128, F * NSEG], F32, isOutput=False)
    mask4_e = nc.declare_dram_parameter("mask4", [1, 4], F32, isOutput=False)
    hm_e = nc.declare_dram_parameter("hm", [1, 1], F32, isOutput=False)
    w_e = nc.declare_dram_parameter("w", [1, C], F32, isOutput=False)
    b_e = nc.declare_dram_parameter("b", [1, C], F32, isOutput=False)
    y_e = nc.declare_dram_parameter("y", [C, TH], BF16, isOutput=True)

    xc_r = xc_e.rearrange("(cb p) t -> p cb t", p=128)
    y_r = y_e.rearrange("(cb p) t -> p cb t", p=128)

    with tile.TileContext(nc) as tc:
        with (
            tc.tile_pool(name="misc", bufs=1) as misc,
            tc.tile_pool(name="xbfp", bufs=NSEG) as xbfp,
            tc.tile_pool(name="absb", bufs=2) as absb,
            tc.tile_pool(name="zpool", bufs=3) as zpool,
            tc.tile_pool(name="xwpool", bufs=2) as xwpool,
            tc.tile_pool(name="tpool", bufs=2) as tpool,
            tc.tile_pool(name="rows", bufs=2) as rows,
            tc.tile_pool(name="tmaj", bufs=2) as tmaj,
            tc.tile_pool(name="cump", bufs=NSEG) as cump,
            tc.tile_pool(name="fin", bufs=2) as fin,
            tc.tile_pool(name="carr", bufs=2) as carr,
            tc.tile_pool(name="dram", bufs=3, space="DRAM") as dram,
            tc.tile_pool(name="pstat", bufs=3, space="PSUM") as pstat,
            tc.tile_pool(name="poffs", bufs=1, space="PSUM") as poffs,
        ):
            # ---- constants
            wjs = []
            for j in range(NTS):
                wj = misc.tile([128, NTS], BF16, tag=f"wj{j}", name=f"wj{j}")
                nc.vector.memset(wj, 0.0)
                nc.vector.memset(wj[:, j : j + 1], 1.0)
                wjs.append(wj)
            ones_f = misc.tile([1, 128], F32, tag="ones_f")
            nc.vector.memset(ones_f, 1.0)
            ones_r = misc.tile([1, 128], F32R, tag="ones_r")
            nc.scalar.copy(out=ones_r, in_=ones_f)
            ones1_f = misc.tile([128, 1], F32, tag="ones1_f")
            nc.vector.memset(ones1_f, 1.0)
            ones1f = misc.tile([128, 1], F32R, tag="ones1f")
            nc.scalar.copy(out=ones1f, in_=ones1_f)
            one11 = misc.tile([1, 1], F32R, tag="one11")
            nc.scalar.copy(out=one11, in_=ones1_f[0:1, :])
            zerosF = misc.tile([128, F], BF16, tag="zerosF")
            nc.vector.memset(zerosF, 0.0)
            eps_t = misc.tile([128, 1], F32, tag="eps_t")
            nc.vector.memset(eps_t, EPS)
            carry0 = misc.tile([1, 2], F32R, tag="carry0")
            nc.scalar.copy(out=carry0, in_=zerosF[0:1, 0:2])
            tri_t = misc.tile([128, 128], F32R, tag="tri_t")
            nc.sync.dma_start(out=tri_t, in_=tri_e[:, :])
            invp_t = misc.tile([128, F * NSEG], F32, tag="invp_t")
            nc.sync.dma_start(out=invp_t, in_=invp_e[:, :])
            invm_t = misc.tile([128, F * NSEG], F32, tag="invm_t")
            nc.sync.dma_start(out=invm_t, in_=invm_e[:, :])
            mask4_t = misc.tile([1, 4], F32, tag="mask4_t")
            nc.sync.dma_start(out=mask4_t, in_=mask4_e[:, :])
            hm_t = misc.tile([1, 1], F32, tag="hm_t")
            nc.sync.dma_start(out=hm_t, in_=hm_e[:, :])
            if wb_general:
                wcol = misc.tile([128, CB], F32, tag="wcol")
                bcol = misc.tile([128, CB], F32, tag="bcol")
                for cb in range(CB):
                    nc.sync.dma_start(
                        out=wcol[:, cb : cb + 1],
                        in_=w_e[0:1, cb * 128 : (cb + 1) * 128].rearrange(
                            "one p -> (one p) 1"
                        ),
                    )
                    nc.sync.dma_start(
                        out=bcol[:, cb : cb + 1],
                        in_=b_e[0:1, cb * 128 : (cb + 1) * 128].rearrange(
                            "one p -> (one p) 1"
                        ),
                    )
            else:
                wdummy = misc.tile([1, C], F32, tag="wdummy")
                nc.sync.dma_start(out=wdummy, in_=w_e[:, :])
                nc.sync.dma_start(out=wdummy, in_=b_e[:, :])

            # CC warm-up + early pair rendezvous: pay the collective
            # stream's init-barrier/first-op cost (~14us) here and bound
            # the pair drift seen by the real boundary AllReduce (which
            # measured ~55us of wait, mostly skew/first-op latency).
            d_w0 = dram.tile([1, 4], F32, tag="d_w0", name="d_w0")
            nc.sync.dma_start(out=d_w0, in_=mask4_e[:, :])
            d_w1 = dram.tile([1, 4], F32, tag="d_w1", name="d_w1")
            nc.gpsimd.collective_compute(
                "AllReduce",
                mybir.AluOpType.add,
                replica_groups=RG,
                ins=[d_w0.opt()],
                outs=[d_w1.opt()],
            )

            carry_cur = carry0

            def stage0(s):
                xbf = xbfp.tile([128, CB, SEG], BF16, tag="xbf")
                nc.sync.dma_start(
                    out=xbf, in_=xc_r[:, :, s * SEG : (s + 1) * SEG]
                )
                return xbf

            def stage1a(s, xbf):
                """Pair-sum channel blocks, then per-t sums via HALF as
                many matmuls: each MM pays ~330ns (LDWEIGHTS flush per MM,
                no dedup in codegen), so contracting (x0+x1) and (x2+x3)
                instead of 4 separate blocks halves PE time. The bf16
                pair-add rounding is well within tolerance (sums of 512
                bf16 values were already the baseline).

                ScalarE squares blocks 0-1; DVE squares blocks 2-3 and
                does all pair-adds (phase-1 DVE is otherwise idle)."""
                JH = NTS // 2
                ps_s = pstat.tile([NTS, TS], F32, tag="ps_s")
                ps_q = pstat.tile([NTS, TS], F32, tag="ps_q")
                for h in range(2):
                    hs = slice(h * HALF, (h + 1) * HALF)
                    xw = xwpool.tile([128, 2, HALF], BF16, tag="xw")
                    for pr, (c0, c1) in enumerate(((0, 1), (2, 3))):
                        nc.vector.tensor_add(
                            out=xw[:, pr, :], in0=xbf[:, c0, hs],
                            in1=xbf[:, c1, hs],
                        )
                    zw = zpool.tile([128, 2, HALF], BF16, tag="zw")
                    t1 = tpool.tile([128, HALF], BF16, tag="t1")
                    t3 = tpool.tile([128, HALF], BF16, tag="t3")
                    nc.scalar.activation(
                        out=zw[:, 0, :], in_=xbf[:, 0, hs],
                        func=mybir.ActivationFunctionType.Square,
                    )
                    nc.scalar.activation(
                        out=t1, in_=xbf[:, 1, hs],
                        func=mybir.ActivationFunctionType.Square,
                    )
                    nc.vector.tensor_add(
                        out=zw[:, 0, :], in0=zw[:, 0, :], in1=t1
                    )
                    nc.scalar.activation(
                        out=zw[:, 1, :], in_=xbf[:, 2, hs],
                        func=mybir.ActivationFunctionType.Square,
                    )
                    nc.vector.tensor_mul(
                        out=t3, in0=xbf[:, 3, hs], in1=xbf[:, 3, hs]
                    )
                    nc.vector.tensor_add(
                        out=zw[:, 1, :], in0=zw[:, 1, :], in1=t3
                    )
                    for jh in range(JH):
                        j = h * JH + jh
                        js = slice(jh * TS, (jh + 1) * TS)
                        for pr in range(2):
                            nc.tensor.matmul(
                                out=ps_s, lhsT=wjs[j], rhs=xw[:, pr, js],
                                start=(j == 0 and pr == 0),
                                stop=(j == NTS - 1 and pr == 1),
                            )
                        for pr in range(2):
                            nc.tensor.matmul(
                                out=ps_q, lhsT=wjs[j], rhs=zw[:, pr, js],
                                start=(j == 0 and pr == 0),
                                stop=(j == NTS - 1 and pr == 1),
                            )
                return ps_s, ps_q

            def stage1b(s, ps_s, ps_q):
                """PSUM export to a local DRAM row pair (no collective)."""
                rows8 = rows.tile([NTS, 2, TS], BF16, tag="rows8")
                with nc.allow_low_precision(reason="bf16 local stat rows"):
                    nc.scalar.copy(out=rows8[:, 0, :], in_=ps_s)
                    nc.scalar.copy(out=rows8[:, 1, :], in_=ps_q)
                d_loc = dram.tile([2, SEG], BF16, tag="d_loc")
                nc.sync.dma_start(
                    out=d_loc.rearrange("q (j c) -> j q c", j=NTS), in_=rows8
                )
                return d_loc

            def stage2a(s, d_loc):
                """Local cumulative sums (t-major scan + running carry)."""
                nonlocal carry_cur
                tm = tmaj.tile([128, 2, F], BF16, tag="tm")
                nc.sync.dma_start(
                    out=tm, in_=d_loc.rearrange("q (p f) -> p q f", p=128)
                )
                tot = tmaj.tile([128, 2, 1], F32R, tag="tot")
                with nc.allow_low_precision(
                    reason="f32r totals feed PE prefix matmuls"
                ):
                    nc.vector.reduce_sum(out=tot, in_=tm, axis=mybir.AxisListType.X)
                offs = poffs.tile([128, 4], F32, tag="offs")
                nc.tensor.matmul(
                    out=offs[:, 0:2], lhsT=ones_r, rhs=carry_cur,
                    start=True, stop=False,
                )
                nc.tensor.matmul(
                    out=offs[:, 0:2], lhsT=tri_t, rhs=tot[:, :, 0],
                    start=False, stop=True,
                )
                nc.tensor.matmul(
                    out=offs[0:1, 2:4], lhsT=ones1f, rhs=tot[:, :, 0],
                    start=True, stop=False,
                )
                nc.tensor.matmul(
                    out=offs[0:1, 2:4], lhsT=one11, rhs=carry_cur,
                    start=False, stop=True,
                )
                carry_new = carr.tile([1, 2], F32R, tag="carry")
                with nc.allow_low_precision(reason="carry feeds PE matmuls"):
                    nc.vector.tensor_copy(out=carry_new, in_=offs[0:1, 2:4])
                carry_cur = carry_new
                cum = cump.tile([128, 2, F], F32, tag="cum")
                for q in range(2):
                    nc.vector.tensor_tensor_scan(
                        out=cum[:, q, :], data0=tm[:, q, :], data1=zerosF,
                        initial=offs[:, q : q + 1],
                        op0=mybir.AluOpType.add, op1=mybir.AluOpType.bypass,
                    )
                return cum

            def boundary():
                """Final carry -> masked [1,4] AllReduce -> os broadcast."""
                cfin = misc.tile([1, 4], F32, tag="cfin")
                nc.scalar.copy(out=cfin[:, 0:2], in_=carry_cur)
                nc.scalar.copy(out=cfin[:, 2:4], in_=carry_cur)
                nc.vector.tensor_mul(out=cfin, in0=cfin, in1=mask4_t)
                d_cin = dram.tile([1, 4], F32, tag="d_cin")
                nc.sync.dma_start(out=d_cin, in_=cfin)
                d_cout = dram.tile([1, 4], F32, tag="d_cout")
                nc.gpsimd.collective_compute(
                    "AllReduce",
                    mybir.AluOpType.add,
                    replica_groups=RG,
                    ins=[d_cin.opt()],
                    outs=[d_cout.opt()],
                )
                r4 = misc.tile([1, 4], F32, tag="r4")
                nc.sync.dma_start(out=r4, in_=d_cout)
                os_f = misc.tile([1, 2], F32, tag="os_f")
                nc.vector.tensor_scalar_mul(
                    out=os_f, in0=r4[:, 0:2], scalar1=hm_t[0:1, 0:1]
                )
                os_r = misc.tile([1, 2], F32R, tag="os_r")
                nc.scalar.copy(out=os_r, in_=os_f)
                osp = poffs.tile([128, 2], F32, tag="osp")
                nc.tensor.matmul(
                    out=osp, lhsT=ones_r, rhs=os_r, start=True, stop=True
                )
                os_b = misc.tile([128, 2], F32, tag="os_b")
                nc.vector.tensor_copy(out=os_b, in_=osp)
                return os_b

            def stage2b(s, cum, os_b):
                """Finalize A/B rows; cross-core offset folds in via
                scalar_tensor_tensor at zero extra op count."""
                invp_s = invp_t[:, s * F : (s + 1) * F]
                invm_s = invm_t[:, s * F : (s + 1) * F]
                nmean = fin.tile([128, F], F32, tag="nmean")
                nc.vector.scalar_tensor_tensor(
                    out=nmean, in0=cum[:, 0, :], scalar=os_b[:, 0:1],
                    in1=invm_s, op0=mybir.AluOpType.add,
                    op1=mybir.AluOpType.mult,
                )
                e2 = fin.tile([128, F], F32, tag="e2")
                nc.vector.scalar_tensor_tensor(
                    out=e2, in0=cum[:, 1, :], scalar=os_b[:, 1:2],
                    in1=invp_s, op0=mybir.AluOpType.add,
                    op1=mybir.AluOpType.mult,
                )
                msq = fin.tile([128, F], F32, tag="msq")
                nc.vector.tensor_mul(out=msq, in0=nmean, in1=nmean)
                var = fin.tile([128, F], F32, tag="var")
                nc.vector.tensor_sub(out=var, in0=e2, in1=msq)
                nc.vector.tensor_scalar_max(out=var, in0=var, scalar1=0.0)
                sd = fin.tile([128, F], F32, tag="sd")
                nc.scalar.activation(
                    out=sd, in_=var, func=mybir.ActivationFunctionType.Sqrt,
                    bias=eps_t, scale=1.0,
                )
                tmo = fin.tile([128, 2, F], BF16, tag="tmo")
                with nc.allow_low_precision(
                    reason="bf16 A/B rows feed the replicated broadcast"
                ):
                    nc.vector.reciprocal(out=tmo[:, 0, :], in_=sd)
                    nc.vector.tensor_mul(
                        out=tmo[:, 1, :], in0=nmean, in1=tmo[:, 0, :]
                    )
                d_ab = dram.tile([2, SEG], BF16, tag="d_ab")
                nc.sync.dma_start(
                    out=d_ab.rearrange("q (p f) -> p q f", p=128), in_=tmo
                )
                # flat stride-0 fan: one DMA per row to all 128 partitions
                a_sb = absb.tile([128, SEG], BF16, tag="a_sb")
                b_sb = absb.tile([128, SEG], BF16, tag="b_sb")
                for row, dst in ((0, a_sb), (1, b_sb)):
                    ap0 = d_ab[row : row + 1, :]
                    src = bass.AP(
                        tensor=ap0.tensor, offset=ap0.offset,
                        ap=[[0, 128], ap0.ap[-1]],
                    )
                    nc.gpsimd.dma_start(out=dst[0:128, :], in_=src)
                return a_sb, b_sb

            def stage3(s, xbf, a_sb, b_sb):
                """y = x*A + B in place, then store. All on DVE: a
                concurrent GpSimd TT on the same partitions was measured
                to halve DVE TT throughput (SBUF port contention). Each
                channel block stores as soon as its two TT passes finish,
                so the store DMA streams behind the DVE instead of waiting
                for the whole segment (shrinks the end-of-kernel tail by
                ~3/4 of a segment store)."""
                for cb in range(CB):
                    xs = xbf[:, cb, :]
                    nc.vector.tensor_mul(out=xs, in0=xs, in1=a_sb)
                    nc.vector.tensor_add(out=xs, in0=xs, in1=b_sb)
                    if wb_general:
                        nc.scalar.activation(
                            out=xs, in_=xs,
                            func=mybir.ActivationFunctionType.Copy,
                            bias=0.0, scale=wcol[:, cb : cb + 1],
                        )
                        nc.vector.tensor_scalar_add(
                            out=xs, in0=xs, scalar1=bcol[:, cb : cb + 1],
                        )
                    nc.sync.dma_start(
                        out=y_r[:, cb : cb + 1, s * SEG : (s + 1) * SEG],
                        in_=xbf[:, cb : cb + 1, :],
                    )

            # Phase 1: fully local, pipelined. xbf tiles for ALL segments
            # stay resident (CB*SEG*NSEG*2B = 128KB/partition) so phase 2
            # never reloads x.
            xbfs, pss, dlocs, cums = {}, {}, {}, {}
            xall = []
            for s in range(NSEG):
                xall.append(stage0(s))
            for it in range(NSEG + 3):
                s1 = it
                if s1 < NSEG:
                    pss[s1] = stage1a(s1, xall[s1])
                s1b = it - 1
                if 0 <= s1b < NSEG:
                    dlocs[s1b] = stage1b(s1b, *pss.pop(s1b))
                if it == 2:
                    # mid-phase rendezvous: bound the pair drift the
                    # boundary AllReduce will see.
                    d_w2 = dram.tile([1, 4], F32, tag="d_w2", name="d_w2")
                    nc.sync.dma_start(out=d_w2, in_=mask4_e[:, :])
                    d_w3 = dram.tile([1, 4], F32, tag="d_w3", name="d_w3")
                    nc.gpsimd.collective_compute(
                        "AllReduce",
                        mybir.AluOpType.add,
                        replica_groups=RG,
                        ins=[d_w2.opt()],
                        outs=[d_w3.opt()],
                    )
                s2 = it - 3
                if 0 <= s2 < NSEG:
                    cums[s2] = stage2a(s2, dlocs.pop(s2))

            os_b = boundary()

            # Phase 2: finalize + broadcast + normalize, pipelined one
            # segment ahead of the consume.
            d_abs = {}
            for it in range(NSEG + 1):
                s2 = it
                if s2 < NSEG:
                    d_abs[s2] = stage2b(s2, cums.pop(s2), os_b)
                s3 = it - 1
                if 0 <= s3 < NSEG:
                    stage3(s3, xall[s3], *d_abs.pop(s3))

    nc.finalize()
    return nc


def _get_kernel(wb_general: bool):
    if wb_general not in _CACHE:
        _CACHE[wb_general] = _build(wb_general)
    return _CACHE[wb_general]


def _make_in_maps(x, weight, bias):
    wb_general = not (np.all(weight == 1.0) and np.all(bias == 0.0))
    tri = np.triu(np.ones((128, 128), np.float32), 1)

    import ml_dtypes

    in_maps = []
    for core in range(NCORES):
        b_idx, h = core // 2, core % 2
        xc = np.ascontiguousarray(
            x[b_idx, :, h * TH : (h + 1) * TH].astype(ml_dtypes.bfloat16)
        )
        # invn[p, s*F + f] = 1 / (C * (t_global + 1)),
        # t_global = h*TH + s*SEG + p*F + f
        t_idx = (
            h * TH
            + np.arange(NSEG)[:, None, None] * SEG
            + np.arange(128)[None, :, None] * F
            + np.arange(F)[None, None, :]
        )
        invn = (1.0 / (C * (t_idx.astype(np.float64) + 1.0))).astype(
            np.float32
        )
        invn = np.ascontiguousarray(
            invn.transpose(1, 0, 2).reshape(128, NSEG * F)
        )
        invm = np.ascontiguousarray(-invn)
        mask4 = np.array(
            [[1.0 - h, 1.0 - h, float(h), float(h)]], np.float32
        )
        hm = np.array([[float(h)]], np.float32)
        w_row = np.ascontiguousarray(
            weight.reshape(1, C).astype(np.float32)
        )
        b_row = np.ascontiguousarray(bias.reshape(1, C).astype(np.float32))
        in_maps.append(
            {
                "xc": xc, "tri": tri, "invp": invn, "invm": invm,
                "mask4": mask4, "hm": hm, "w": w_row, "b": b_row,
            }
        )
    return in_maps, wb_general


def kernel(x, weight, bias, _trace=False, _tmpdir=None):
    x = np.asarray(x, np.float32)
    weight = np.asarray(weight, np.float32)
    bias = np.asarray(bias, np.float32)
    in_maps, wb_general = _make_in_maps(x, weight, bias)
    nc = _get_kernel(wb_general)
    res = run_bass_kernel_spmd(
        nc, in_maps, list(range(NCORES)), trace=_trace, tmpdir=_tmpdir
    )
    y = np.empty((B, C, T), np.float32)
    for core in range(NCORES):
        b_idx, h = core // 2, core % 2
        y[b_idx, :, h * TH : (h + 1) * TH] = res.results[core]["y"].astype(
            np.float32
        )
    if _trace:
        return y, res
    return y


# revision 30
# speedup vs baseline: 1.0216x; 1.0216x over previous
"""Cumulative LayerNorm (B=4, C=512, T=32000) on 8 Trainium2 NeuronCores.

Sharding: core j handles batch b = j//2 and T-half h = j%2 (16000 time
steps), ALL 512 channels. Per-t channel sums are complete locally, so
the only cross-core data is each half's grand total (2 f32 scalars):
ONE 16-byte AllReduce per kernel, vs. the channel-split design's 128KB
through the ~2GB/s CC engine (which measured ~10us/op serialized and
paced the whole kernel).

x is fed bf16 and y stored bf16 (upcast on host).

Structure per core:
  Phase 1 (pipelined over 5 x 3200-t segments): load | squares
    (ScalarE 2cb + DVE 2cb) | per-t sums via one-hot-column matmuls
    into [8,400] PSUM banks | PSUM export | local t-major scan with
    running carry (offsets via strict-triangular PE matmul).
  Boundary: final carry -> masked [1,4] f32 AllReduce (even core's
    totals land in slots 0:2) -> os = r4[0:2]*h -> PE-broadcast os to
    all 128 partitions.
  Phase 2 (pipelined): finalize A/B per segment -- the cross-core
    offset folds into the existing ops via scalar_tensor_tensor
    ((cum+os)*inv) at zero extra cost -- then flat 128-partition
    stride-0 fan of the A/B rows, y = x*A + B (DVE 3cb + GpSimd 1cb),
    store.
"""
import numpy as np

import concourse.bass as bass
import concourse.bacc as bacc
import concourse.tile as tile
from concourse import mybir
from concourse.bass_utils import run_bass_kernel_spmd

F32 = mybir.dt.float32
F32R = mybir.dt.float32r
BF16 = mybir.dt.bfloat16

B, C, T = 4, 512, 32000
NCORES = 8
TH = T // 2          # 16000 t per core
CB = C // 128        # 4 channel blocks
SEG = 3200           # segment length along T
NSEG = TH // SEG     # 5
F = SEG // 128       # 25 (t-major free dim per segment)
TS = 400             # stats matmul tile (moving cols)
NTS = SEG // TS      # 8
HALF = SEG // 2
EPS = 1e-08
RG = [[0, 1], [2, 3], [4, 5], [6, 7]]  # batch-pair replica groups

_CACHE = {}


def _build(wb_general: bool):
    nc = bacc.Bacc()

    xc_e = nc.declare_dram_parameter("xc", [C, TH], BF16, isOutput=False)
    tri_e = nc.declare_dram_parameter("tri", [128, 128], F32R, isOutput=False)
    invp_e = nc.declare_dram_parameter("invp", [128, F * NSEG], F32, isOutput=False)
    invm_e = nc.declare_dram_parameter("invm", [128, F * NSEG], F32, isOutput=False)
    mask4_e = nc.declare_dram_parameter("mask4", [1, 4], F32, isOutput=False)
    hm_e = nc.declare_dram_parameter("hm", [1, 1], F32, isOutput=False)
    w_e = nc.declare_dram_parameter("w", [1, C], F32, isOutput=False)
    b_e = nc.declare_dram_parameter("b", [1, C], F32, isOutput=False)
    y_e = nc.declare_dram_parameter("y", [C, TH], BF16, isOutput=True)

    xc_r = xc_e.rearrange("(cb p) t -> p cb t", p=128)
    y_r = y_e.rearrange("(cb p) t -> p cb t", p=128)

    with tile.TileContext(nc) as tc:
        with (
            tc.tile_pool(name="misc", bufs=1) as misc,
            tc.tile_pool(name="xbfp", bufs=NSEG) as xbfp,
            tc.tile_pool(name="absb", bufs=2) as absb,
            tc.tile_pool(name="zpool", bufs=3) as zpool,
            tc.tile_pool(name="xwpool", bufs=2) as xwpool,
            tc.tile_pool(name="tpool", bufs=2) as tpool,
            tc.tile_pool(name="rows", bufs=2) as rows,
            tc.tile_pool(name="tmaj", bufs=2) as tmaj,
            tc.tile_pool(name="cump", bufs=NSEG) as cump,
            tc.tile_pool(name="fin", bufs=2) as fin,
            tc.tile_pool(name="carr", bufs=2) as carr,
            tc.tile_pool(name="dram", bufs=3, space="DRAM") as dram,
            tc.tile_pool(name="pstat", bufs=3, space="PSUM") as pstat,
            tc.tile_pool(name="poffs", bufs=1, space="PSUM") as poffs,
        ):
            # ---- constants
            wjs = []
            for j in range(NTS):
                wj = misc.tile([128, NTS], BF16, tag=f"wj{j}", name=f"wj{j}")
                nc.vector.memset(wj, 0.0)
                nc.vector.memset(wj[:, j : j + 1], 1.0)
                wjs.append(wj)
            ones_f = misc.tile([1, 128], F32, tag="ones_f")
            nc.vector.memset(ones_f, 1.0)
            ones_r = misc.tile([1, 128], F32R, tag="ones_r")
            nc.scalar.copy(out=ones_r, in_=ones_f)
            ones1_f = misc.tile([128, 1], F32, tag="ones1_f")
            nc.vector.memset(ones1_f, 1.0)
            ones1f = misc.tile([128, 1], F32R, tag="ones1f")
            nc.scalar.copy(out=ones1f, in_=ones1_f)
            one11 = misc.tile([1, 1], F32R, tag="one11")
            nc.scalar.copy(out=one11, in_=ones1_f[0:1, :])
            zerosF = misc.tile([128, F], BF16, tag="zerosF")
            nc.vector.memset(zerosF, 0.0)
            eps_t = misc.tile([128, 1], F32, tag="eps_t")
            nc.vector.memset(eps_t, EPS)
            carry0 = misc.tile([1, 2], F32R, tag="carry0")
            nc.scalar.copy(out=carry0, in_=zerosF[0:1, 0:2])
            tri_t = misc.tile([128, 128], F32R, tag="tri_t")
            nc.sync.dma_start(out=tri_t, in_=tri_e[:, :])
            invp_t = misc.tile([128, F * NSEG], F32, tag="invp_t")
            nc.sync.dma_start(out=invp_t, in_=invp_e[:, :])
            invm_t = misc.tile([128, F * NSEG], F32, tag="invm_t")
            nc.sync.dma_start(out=invm_t, in_=invm_e[:, :])
            mask4_t = misc.tile([1, 4], F32, tag="mask4_t")
            nc.sync.dma_start(out=mask4_t, in_=mask4_e[:, :])
            hm_t = misc.tile([1, 1], F32, tag="hm_t")
            nc.sync.dma_start(out=hm_t, in_=hm_e[:, :])
            if wb_general:
                wcol = misc.tile([128, CB], F32, tag="wcol")
                bcol = misc.tile([128, CB], F32, tag="bcol")
                for cb in range(CB):
                    nc.sync.dma_start(
                        out=wcol[:, cb : cb + 1],
                        in_=w_e[0:1, cb * 128 : (cb + 1) * 128].rearrange(
                            "one p -> (one p) 1"
                        ),
                    )
                    nc.sync.dma_start(
                        out=bcol[:, cb : cb + 1],
                        in_=b_e[0:1, cb * 128 : (cb + 1) * 128].rearrange(
                            "one p -> (one p) 1"
                        ),
                    )
            else:
                wdummy = misc.tile([1, C], F32, tag="wdummy")
                nc.sync.dma_start(out=wdummy, in_=w_e[:, :])
                nc.sync.dma_start(out=wdummy, in_=b_e[:, :])

            # CC warm-up + early pair rendezvous: pay the collective
            # stream's init-barrier/first-op cost (~14us) here and bound
            # the pair drift seen by the real boundary AllReduce (which
            # measured ~55us of wait, mostly skew/first-op latency).
            d_w0 = dram.tile([1, 4], F32, tag="d_w0", name="d_w0")
            nc.sync.dma_start(out=d_w0, in_=mask4_e[:, :])
            d_w1 = dram.tile([1, 4], F32, tag="d_w1", name="d_w1")
            nc.gpsimd.collective_compute(
                "AllReduce",
                mybir.AluOpType.add,
                replica_groups=RG,
                ins=[d_w0.opt()],
                outs=[d_w1.opt()],
            )

            carry_cur = carry0

            def stage0(s):
                xbf = xbfp.tile([128, CB, SEG], BF16, tag="xbf")
                nc.sync.dma_start(
                    out=xbf, in_=xc_r[:, :, s * SEG : (s + 1) * SEG]
                )
                return xbf

            def stage1a(s, xbf):
                """Pair-sum channel blocks, then per-t sums via HALF as
                many matmuls: each MM pays ~330ns (LDWEIGHTS flush per MM,
                no dedup in codegen), so contracting (x0+x1) and (x2+x3)
                instead of 4 separate blocks halves PE time. The bf16
                pair-add rounding is well within tolerance (sums of 512
                bf16 values were already the baseline).

                ScalarE squares blocks 0-1; DVE squares blocks 2-3 and
                does all pair-adds (phase-1 DVE is otherwise idle)."""
                JH = NTS // 2
                ps_s = pstat.tile([NTS, TS], F32, tag="ps_s")
                ps_q = pstat.tile([NTS, TS], F32, tag="ps_q")
                for h in range(2):
                    hs = slice(h * HALF, (h + 1) * HALF)
                    xw = xwpool.tile([128, 2, HALF], BF16, tag="xw")
                    for pr, (c0, c1) in enumerate(((0, 1), (2, 3))):
                        nc.vector.tensor_add(
                            out=xw[:, pr, :], in0=xbf[:, c0, hs],
                            in1=xbf[:, c1, hs],
                        )
                    zw = zpool.tile([128, 2, HALF], BF16, tag="zw")
                    t1 = tpool.tile([128, HALF], BF16, tag="t1")
                    t3 = tpool.tile([128, HALF], BF16, tag="t3")
                    nc.scalar.activation(
                        out=zw[:, 0, :], in_=xbf[:, 0, hs],
                        func=mybir.ActivationFunctionType.Square,
                    )
                    nc.scalar.activation(
                        out=t1, in_=xbf[:, 1, hs],
                        func=mybir.ActivationFunctionType.Square,
                    )
                    nc.vector.tensor_add(
                        out=zw[:, 0, :], in0=zw[:, 0, :], in1=t1
                    )
                    nc.scalar.activation(
                        out=zw[:, 1, :], in_=xbf[:, 2, hs],
                        func=mybir.ActivationFunctionType.Square,
                    )
                    nc.vector.tensor_mul(
                        out=t3, in0=xbf[:, 3, hs], in1=xbf[:, 3, hs]
                    )
                    nc.vector.tensor_add(
                        out=zw[:, 1, :], in0=zw[:, 1, :], in1=t3
                    )
                    for jh in range(JH):
                        j = h * JH + jh
                        js = slice(jh * TS, (jh + 1) * TS)
                        for pr in range(2):
                            nc.tensor.matmul(
                                out=ps_s, lhsT=wjs[j], rhs=xw[:, pr, js],
                                start=(j == 0 and pr == 0),
                                stop=(j == NTS - 1 and pr == 1),
                            )
                        for pr in range(2):
                            nc.tensor.matmul(
                                out=ps_q, lhsT=wjs[j], rhs=zw[:, pr, js],
                                start=(j == 0 and pr == 0),
                                stop=(j == NTS - 1 and pr == 1),
                            )
                return ps_s, ps_q

            def stage1b(s, ps_s, ps_q):
                """PSUM export to a local DRAM row pair (no collective)."""
                rows8 = rows.tile([NTS, 2, TS], BF16, tag="rows8")
                with nc.allow_low_precision(reason="bf16 local stat rows"):
                    nc.scalar.copy(out=rows8[:, 0, :], in_=ps_s)
                    nc.scalar.copy(out=rows8[:, 1, :], in_=ps_q)
                d_loc = dram.tile([2, SEG], BF16, tag="d_loc")
                nc.sync.dma_start(
                    out=d_loc.rearrange("q (j c) -> j q c", j=NTS), in_=rows8
                )
                return d_loc

            def stage2a(s, d_loc):
                """Local cumulative sums (t-major scan + running carry)."""
                nonlocal carry_cur
                tm = tmaj.tile([128, 2, F], BF16, tag="tm")
                nc.sync.dma_start(
                    out=tm, in_=d_loc.rearrange("q (p f) -> p q f", p=128)
                )
                tot = tmaj.tile([128, 2, 1], F32R, tag="tot")
                with nc.allow_low_precision(
                    reason="f32r totals feed PE prefix matmuls"
                ):
                    nc.vector.reduce_sum(out=tot, in_=tm, axis=mybir.AxisListType.X)
                offs = poffs.tile([128, 4], F32, tag="offs")
                nc.tensor.matmul(
                    out=offs[:, 0:2], lhsT=ones_r, rhs=carry_cur,
                    start=True, stop=False,
                )
                nc.tensor.matmul(
                    out=offs[:, 0:2], lhsT=tri_t, rhs=tot[:, :, 0],
                    start=False, stop=True,
                )
                nc.tensor.matmul(
                    out=offs[0:1, 2:4], lhsT=ones1f, rhs=tot[:, :, 0],
                    start=True, stop=False,
                )
                nc.tensor.matmul(
                    out=offs[0:1, 2:4], lhsT=one11, rhs=carry_cur,
                    start=False, stop=True,
                )
                carry_new = carr.tile([1, 2], F32R, tag="carry")
                with nc.allow_low_precision(reason="carry feeds PE matmuls"):
                    nc.vector.tensor_copy(out=carry_new, in_=offs[0:1, 2:4])
                carry_cur = carry_new
                cum = cump.tile([128, 2, F], F32, tag="cum")
                for q in range(2):
                    nc.vector.tensor_tensor_scan(
                        out=cum[:, q, :], data0=tm[:, q, :], data1=zerosF,
                        initial=offs[:, q : q + 1],
                        op0=mybir.AluOpType.add, op1=mybir.AluOpType.bypass,
                    )
                return cum

            def boundary():
                """Final carry -> masked [1,4] AllReduce -> os broadcast."""
                cfin = misc.tile([1, 4], F32, tag="cfin")
                nc.scalar.copy(out=cfin[:, 0:2], in_=carry_cur)
                nc.scalar.copy(out=cfin[:, 2:4], in_=carry_cur)
                nc.vector.tensor_mul(out=cfin, in0=cfin, in1=mask4_t)
                d_cin = dram.tile([1, 4], F32, tag="d_cin")
                nc.sync.dma_start(out=d_cin, in_=cfin)
                d_cout = dram.tile([1, 4], F32, tag="d_cout")
                nc.gpsimd.collective_compute(
                    "AllReduce",
                    mybir.AluOpType.add,
                    replica_groups=RG,
                    ins=[d_cin.opt()],
                    outs=[d_cout.opt()],
                )
                r4 = misc.tile([1, 4], F32, tag="r4")
                nc.sync.dma_start(out=r4, in_=d_cout)
                os_f = misc.tile([1, 2], F32, tag="os_f")
                nc.vector.tensor_scalar_mul(
                    out=os_f, in0=r4[:, 0:2], scalar1=hm_t[0:1, 0:1]
                )
                os_r = misc.tile([1, 2], F32R, tag="os_r")
                nc.scalar.copy(out=os_r, in_=os_f)
                osp = poffs.tile([128, 2], F32, tag="osp")
                nc.tensor.matmul(
                    out=osp, lhsT=ones_r, rhs=os_r, start=True, stop=True
                )
                os_b = misc.tile([128, 2], F32, tag="os_b")
                nc.vector.tensor_copy(out=os_b, in_=osp)
                return os_b

            def stage2b(s, cum, os_b):
                """Finalize A/B rows; cross-core offset folds in via
                scalar_tensor_tensor at zero extra op count."""
                invp_s = invp_t[:, s * F : (s + 1) * F]
                invm_s = invm_t[:, s * F : (s + 1) * F]
                nmean = fin.tile([128, F], F32, tag="nmean")
                nc.vector.scalar_tensor_tensor(
                    out=nmean, in0=cum[:, 0, :], scalar=os_b[:, 0:1],
                    in1=invm_s, op0=mybir.AluOpType.add,
                    op1=mybir.AluOpType.mult,
                )
                e2 = fin.tile([128, F], F32, tag="e2")
                nc.vector.scalar_tensor_tensor(
                    out=e2, in0=cum[:, 1, :], scalar=os_b[:, 1:2],
                    in1=invp_s, op0=mybir.AluOpType.add,
                    op1=mybir.AluOpType.mult,
                )
                msq = fin.tile([128, F], F32, tag="msq")
                nc.vector.tensor_mul(out=msq, in0=nmean, in1=nmean)
                var = fin.tile([128, F], F32, tag="var")
                nc.vector.tensor_sub(out=var, in0=e2, in1=msq)
                nc.vector.tensor_scalar_max(out=var, in0=var, scalar1=0.0)
                sd = fin.tile([128, F], F32, tag="sd")
                nc.scalar.activation(
                    out=sd, in_=var, func=mybir.ActivationFunctionType.Sqrt,
                    bias=eps_t, scale=1.0,
                )
                tmo = fin.tile([128, 2, F], BF16, tag="tmo")
                with nc.allow_low_precision(
                    reason="bf16 A/B rows feed the replicated broadcast"
                ):
                    nc.vector.reciprocal(out=tmo[:, 0, :], in_=sd)
                    nc.vector.tensor_mul(
                        out=tmo[:, 1, :], in0=nmean, in1=tmo[:, 0, :]
                    )
                d_ab = dram.tile([2, SEG], BF16, tag="d_ab")
                nc.sync.dma_start(
                    out=d_ab.rearrange("q (p f) -> p q f", p=128), in_=tmo
                )
                # flat stride-0 fan: one DMA per row to all 128 partitions
                a_sb = absb.tile([128, SEG], BF16, tag="a_sb")
                b_sb = absb.tile([128, SEG], BF16, tag="b_sb")
                for row, dst in ((0, a_sb), (1, b_sb)):
                    ap0 = d_ab[row : row + 1, :]
                    src = bass.AP(
                        tensor=ap0.tensor, offset=ap0.offset,
                        ap=[[0, 128], ap0.ap[-1]],
                    )
                    nc.gpsimd.dma_start(out=dst[0:128, :], in_=src)
                return a_sb, b_sb

            def stage3(s, xbf, a_sb, b_sb):
                """y = x*A + B in place, then store. All on DVE: a
                concurrent GpSimd TT on the same partitions was measured
                to halve DVE TT throughput (SBUF port contention)."""
                for cb in range(CB):
                    xs = xbf[:, cb, :]
                    nc.vector.tensor_mul(out=xs, in0=xs, in1=a_sb)
                    nc.vector.tensor_add(out=xs, in0=xs, in1=b_sb)
                    if wb_general:
                        nc.scalar.activation(
                            out=xs, in_=xs,
                            func=mybir.ActivationFunctionType.Copy,
                            bias=0.0, scale=wcol[:, cb : cb + 1],
                        )
                        nc.vector.tensor_scalar_add(
                            out=xs, in0=xs, scalar1=bcol[:, cb : cb + 1],
                        )
                nc.sync.dma_start(
                    out=y_r[:, :, s * SEG : (s + 1) * SEG], in_=xbf
                )

            # Phase 1: fully local, pipelined. xbf tiles for ALL segments
            # stay resident (CB*SEG*NSEG*2B = 128KB/partition) so phase 2
            # never reloads x.
            xbfs, pss, dlocs, cums = {}, {}, {}, {}
            xall = []
            for s in range(NSEG):
                xall.append(stage0(s))
            for it in range(NSEG + 3):
                s1 = it
                if s1 < NSEG:
                    pss[s1] = stage1a(s1, xall[s1])
                s1b = it - 1
                if 0 <= s1b < NSEG:
                    dlocs[s1b] = stage1b(s1b, *pss.pop(s1b))
                if it == 2:
                    # mid-phase rendezvous: bound the pair drift the
                    # boundary AllReduce will see.
                    d_w2 = dram.tile([1, 4], F32, tag="d_w2", name="d_w2")
                    nc.sync.dma_start(out=d_w2, in_=mask4_e[:, :])
                    d_w3 = dram.tile([1, 4], F32, tag="d_w3", name="d_w3")
                    nc.gpsimd.collective_compute(
                        "AllReduce",
                        mybir.AluOpType.add,
                        replica_groups=RG,
                        ins=[d_w2.opt()],
                        outs=[d_w3.opt()],
                    )
                s2 = it - 3
                if 0 <= s2 < NSEG:
                    cums[s2] = stage2a(s2, dlocs.pop(s2))

            os_b = boundary()

            # Phase 2: finalize + broadcast + normalize, pipelined one
            # segment ahead of the consume.
            d_abs = {}
            for it in range(NSEG + 1):
                s2 = it
                if s2 < NSEG:
                    d_abs[s2] = stage2b(s2, cums.pop(s2), os_b)
                s3 = it - 1
                if 0 <= s3 < NSEG:
                    stage3(s3, xall[s3], *d_abs.pop(s3))

    nc.finalize()
    return nc


def _get_kernel(wb_general: bool):
    if wb_general not in _CACHE:
        _CACHE[wb_general] = _build(wb_general)
    return _CACHE[wb_general]


def _make_in_maps(x, weight, bias):
    wb_general = not (np.all(weight == 1.0) and np.all(bias == 0.0))
    tri = np.triu(np.ones((128, 128), np.float32), 1)

    import ml_dtypes

    in_maps = []
    for core in range(NCORES):
        b_idx, h = core // 2, core % 2
        xc = np.ascontiguousarray(
            x[b_idx, :, h * TH : (h + 1) * TH].astype(ml_dtypes.bfloat16)
        )
        # invn[p, s*F + f] = 1 / (C * (t_global + 1)),
        # t_global = h*TH + s*SEG + p*F + f
        t_idx = (
            h * TH
            + np.arange(NSEG)[:, None, None] * SEG
            + np.arange(128)[None, :, None] * F
            + np.arange(F)[None, None, :]
        )
        invn = (1.0 / (C * (t_idx.astype(np.float64) + 1.0))).astype(
            np.float32
        )
        invn = np.ascontiguousarray(
            invn.transpose(1, 0, 2).reshape(128, NSEG * F)
        )
        invm = np.ascontiguousarray(-invn)
        mask4 = np.array(
            [[1.0 - h, 1.0 - h, float(h), float(h)]], np.float32
        )
        hm = np.array([[float(h)]], np.float32)
        w_row = np.ascontiguousarray(
            weight.reshape(1, C).astype(np.float32)
        )
        b_row = np.ascontiguousarray(bias.reshape(1, C).astype(np.float32))
        in_maps.append(
            {
                "xc": xc, "tri": tri, "invp": invn, "invm": invm,
                "mask4": mask4, "hm": hm, "w": w_row, "b": b_row,
            }
        )
    return in_maps, wb_general


def kernel(x, weight, bias, _trace=False, _tmpdir=None):
    x = np.asarray(x, np.float32)
    weight = np.asarray(weight, np.float32)
    bias = np.asarray(bias, np.float32)
    in_maps, wb_general = _make_in_maps(x, weight, bias)
    nc = _get_kernel(wb_general)
    res = run_bass_kernel_spmd(
        nc, in_maps, list(range(NCORES)), trace=_trace, tmpdir=_tmpdir
    )
    y = np.empty((B, C, T), np.float32)
    for core in range(NCORES):
        b_idx, h = core // 2, core % 2
        y[b_idx, :, h * TH : (h + 1) * TH] = res.results[core]["y"].astype(
            np.float32
        )
    if _trace:
        return y, res
    return y
